# revision 42
# baseline (speedup 1.0000x reference)
"""Multi-head causal attention on 8 Trainium2 NeuronCores.

Problem: B=2, S=2048, D=1024, H=16, DH=64, causal mask, f32.

Sharding: core c -> (batch b = c//4, head group g = c%4 of 4 heads).
Each core computes Q/K/V projections for its 4 heads, streamed causal
attention, AllGathers Z across the 4 cores of a batch group per q-chunk,
and computes a 256-column slice of the output projection.  Host
concatenates slices.

v2 schedule: q-chunks processed ASCENDING (j=0 first) so the exp stream
starts ~20us in and each chunk's AllGather is issued as early as
possible.  Projections for chunk j+1 and the O-projection for chunk j-1
are interleaved into chunk j's attention pair stream so the PE never
starves while ACT (exp) is the constraint.  The final chunk's gather is
split in two (by head pair) so only ~half of it is exposed as tail
latency.  Softmax 1/r broadcast runs on the idle GpSimd engine instead
of a 4-DMA round trip through DRAM.
"""
import os
import numpy as np
import ml_dtypes
from contextlib import ExitStack

import concourse.bacc as bacc
import concourse.tile as tile
from concourse import mybir
from concourse import bass_utils

F32 = mybir.dt.float32
BF16 = mybir.dt.bfloat16
AF = mybir.ActivationFunctionType

B, S, D, H, DH = 2, 2048, 1024, 16, 64
NCORES = 8
HLOC = 4            # heads per core
QC = 512            # q chunk width
NQ = S // QC        # 4 q chunks
KT = 128            # k tile height
NKT = S // KT       # 16 k tiles
MC = D // 128       # 8 contraction chunks
NSL = D // 4        # 256 output columns per core
INV_SCALE = 1.0 / float(np.sqrt(DH))

_cache = {}


def _build():
    nc = bacc.Bacc("TRN2", target_bir_lowering=False, debug=False,
                   num_devices=NCORES)

    xT_d = nc.dram_tensor("xT", [D, S], BF16, kind="ExternalInput").ap()
    wq_d = nc.dram_tensor("wq", [D, 256], BF16, kind="ExternalInput").ap()
    wk_d = nc.dram_tensor("wk", [D, 256], BF16, kind="ExternalInput").ap()
    wv_d = nc.dram_tensor("wv", [D, 256], BF16, kind="ExternalInput").ap()
    wo_d = nc.dram_tensor("wo", [D, NSL], BF16, kind="ExternalInput").ap()
    bqk_d = nc.dram_tensor("bqk", [128, 4], F32, kind="ExternalInput").ap()
    bo_d = nc.dram_tensor("bo", [128, 2], F32, kind="ExternalInput").ap()
    triu_d = nc.dram_tensor("triu", [128, 128], BF16, kind="ExternalInput").ap()
    out_d = nc.dram_tensor("outT", [NSL, S], F32, kind="ExternalOutput").ap()

    with tile.TileContext(nc) as tc, ExitStack() as ctx:
        singles = ctx.enter_context(tc.tile_pool(name="singles", bufs=1))
        ptpool = ctx.enter_context(tc.tile_pool(name="pt", bufs=3))
        ztpool = ctx.enter_context(tc.tile_pool(name="zt", bufs=6))
        rpool = ctx.enter_context(tc.tile_pool(name="rp", bufs=12))
        opool = ctx.enter_context(tc.tile_pool(name="op", bufs=2))
        zapool = ctx.enter_context(tc.tile_pool(name="za", bufs=10))
        ps_pool = ctx.enter_context(tc.tile_pool(name="ps", bufs=3, space="PSUM"))
        pr_pool = ctx.enter_context(tc.tile_pool(name="pr", bufs=2, space="PSUM"))
        dram = ctx.enter_context(tc.tile_pool(name="dram", bufs=1, space="DRAM"))

        # ---------------- persistent SBUF tensors ----------------
        xt_sb = singles.tile([128, MC, S], BF16)      # x[b].T, m-chunked
        wq_sb = singles.tile([128, MC, 256], BF16)
        wk_sb = singles.tile([128, MC, 256], BF16)
        wv_sb = singles.tile([128, MC, 256], BF16)
        wo_sb = singles.tile([128, MC, NSL], BF16)
        bqk_sb = singles.tile([128, 4], F32)
        bo_sb = singles.tile([128, 2], F32)
        triu_sb = singles.tile([128, 128], BF16)
        ones_col = singles.tile([128, 1], F32)
        # Per-chunk tensors (separate tiles -> clean dependency tracking)
        # Q^T chunk: [128, hp, 512]; head pairs packed in partitions.
        qt_c = [singles.tile([128, 2, QC], BF16, name=f"qt{j}")
                for j in range(NQ)]
        # K^T zero-padded per head: even head h keeps rows 0..63 (rest 0),
        # odd head keeps rows 64..127, so the stationary is [128,128] and
        # the PE pipelines LDWEIGHTS (FWL) instead of serializing it.
        ktz_c = [singles.tile([128, HLOC, QC], BF16, name=f"ktz{j}")
                 for j in range(NQ)]
        # V' layout per ktile: [V_h | 1] x 4 heads (65 cols each), padded to
        # 324 so every head can present a [128, 128] stationary slice
        v_c = [singles.tile([128, 4, 324], BF16, name=f"v{j}")
               for j in range(NQ)]

        # ---------------- input DMAs ----------------
        # Keep the scalar (ACT) queue empty so the exp stream starts early;
        # spread issues across sync and gpsimd.
        # The sync queue carries only the small weight loads (so wk/wq land
        # in ~3us and the K/Q projections start immediately); the bulky x^T
        # stream goes on the otherwise-idle gpsimd queue, column-chunk-major
        # (chunk jj's projections need only columns [jj*QC, (jj+1)*QC) of
        # every m-chunk, so the first 4 DMAs unblock chunk 0).
        xT_r = xT_d.rearrange("(c p) q -> p c q", p=128)
        nc.sync.dma_start(wk_sb[:],
                          wk_d.rearrange("(c p) hd -> p c hd", p=128))
        nc.sync.dma_start(bqk_sb[:], bqk_d[:])
        nc.sync.dma_start(wq_sb[:],
                          wq_d.rearrange("(c p) hd -> p c hd", p=128))
        nc.sync.dma_start(triu_sb[:], triu_d[:])
        nc.gpsimd.dma_start(wv_sb[:],
                            wv_d.rearrange("(c p) hd -> p c hd", p=128))
        for jj in range(NQ):
            for mc in range(0, MC, 2):
                nc.gpsimd.dma_start(
                    xt_sb[:, mc:mc + 2, jj * QC:(jj + 1) * QC],
                    xT_r[:, mc:mc + 2, jj * QC:(jj + 1) * QC])
        nc.gpsimd.dma_start(bo_sb[:], bo_d[:])
        nc.gpsimd.dma_start(wo_sb[:],
                            wo_d.rearrange("(c p) n -> p c n", p=128))

        # PE warm-up: the first projections are DMA-paced and too sparse to
        # lift the HAM clock gate out of its cold 1.2 GHz state, so burn a
        # dense burst of dummy matmuls while x^T streams in.  ~40 N=128
        # matmuls ≈ 4us of sustained PE-busy -> K=8/8 before K-proj starts.
        warm_sb = singles.tile([128, 128], BF16)
        nc.vector.memset(warm_sb[:], 0.0)
        warm_ps = pr_pool.tile([128, 128], F32, tag="pr", name="warm")
        for _ in range(40):
            nc.tensor.matmul(warm_ps[:], warm_sb[:], warm_sb[:],
                             start=True, stop=True)

        nc.vector.memset(ones_col[:], 1.0)
        for jj in range(NQ):
            nc.vector.memset(v_c[jj][:], 0.0)
            for h in range(HLOC):
                if h % 2 == 0:
                    nc.vector.memset(ktz_c[jj][64:128, h, :], 0.0)
                else:
                    nc.vector.memset(ktz_c[jj][0:64, h, :], 0.0)
        for jj in range(NQ):
            ones_v = v_c[jj][:, :, 0:260].rearrange("p k (h c) -> p k h c", c=65)
            for kt in range(4):
                nc.vector.tensor_copy(ones_v[:, kt, :, 64],
                                      ones_col[:, 0:1].to_broadcast((128, 4)))

        # ---------------- projection group emitters ----------------
        # Each projection group is split into two 4-matmul halves so the
        # in-order PE queue never blocks the attention stream for more than
        # ~1us at a time.
        pp_live = {}

        def emit_qk_group(w_sb, is_k, bcol, hp, jj, half):
            # one [128, 512] output chunk of Q^T or K^T (2 heads packed)
            key = (is_k, hp, jj)
            if half == 0:
                pp_live[key] = pr_pool.tile([128, QC], F32, tag="pr",
                                            name=f"pp{int(is_k)}{hp}{jj}")
            pp = pp_live[key]
            for mc in range(4 * half, 4 * half + 4):
                nc.tensor.matmul(
                    pp[:],
                    w_sb[:, mc, hp * 128:(hp + 1) * 128],
                    xt_sb[:, mc, jj * QC:(jj + 1) * QC],
                    start=(mc == 0), stop=(mc == MC - 1))
            if half == 0:
                return
            del pp_live[key]
            if not is_k:
                nc.vector.tensor_scalar_add(
                    qt_c[jj][:, hp, :],
                    pp[:], bqk_sb[:, bcol + hp:bcol + hp + 1])
            else:
                nc.vector.tensor_scalar_add(
                    ktz_c[jj][0:64, 2 * hp, :],
                    pp[0:64, :], bqk_sb[0:64, bcol + hp:bcol + hp + 1])
                nc.vector.tensor_scalar_add(
                    ktz_c[jj][64:128, 2 * hp + 1, :],
                    pp[64:128, :],
                    bqk_sb[64:128, bcol + hp:bcol + hp + 1])

        def emit_v_group(i, half):
            # V[k, hd] for k tile i (natural layout; bias folded into b_O)
            jj, kt = i // 4, i % 4
            if half == 0:
                pp_live[("v", i)] = pr_pool.tile([128, 256], F32, tag="pr",
                                                 name=f"vp{i}")
            vp = pp_live[("v", i)]
            for mc in range(4 * half, 4 * half + 4):
                nc.tensor.matmul(
                    vp[:],
                    xt_sb[:, mc, i * 128:(i + 1) * 128],
                    wv_sb[:, mc, :],
                    start=(mc == 0), stop=(mc == MC - 1))
            if half == 0:
                return
            del pp_live[("v", i)]
            ones_v = v_c[jj][:, :, 0:260].rearrange("p k (h c) -> p k h c", c=65)
            nc.vector.tensor_copy(
                ones_v[:, kt, :, 0:64],
                vp[:].rearrange("p (h c) -> p h c", c=64))

        def proj_groups(jj):
            gs = []
            for hp in range(2):
                for half in range(2):
                    gs.append(lambda hp=hp, half=half:
                              emit_qk_group(wk_sb, True, 2, hp, jj, half))
            for hp in range(2):
                for half in range(2):
                    gs.append(lambda hp=hp, half=half:
                              emit_qk_group(wq_sb, False, 0, hp, jj, half))
            for i in range(4 * jj, 4 * jj + 4):
                for half in range(2):
                    gs.append(lambda i=i, half=half: emit_v_group(i, half))
            return gs

        # ---------------- DRAM staging for Z + collectives ----------------
        zt_b = [dram.tile([HLOC * 64, QC], BF16, name=f"ztb{j}")
                for j in range(3)]
        zt_all = [dram.tile([H * 64, QC], BF16, name=f"zta{j}")
                  for j in range(3)]
        ztb3 = [dram.tile([128, QC], BF16, name=f"ztb3{u}") for u in range(2)]
        zta3 = [dram.tile([512, QC], BF16, name=f"zta3{u}") for u in range(2)]
        r_dram = [dram.tile([1, QC], F32, name=f"rd{j}_{h}")
                  for j in range(NQ) for h in range(HLOC)]
        r_dram2 = [dram.tile([1, QC], F32, name=f"re{j}_{h}")
                   for j in range(NQ) for h in range(HLOC)]

        def emit_gather(src_t, dst_t):
            nc.gpsimd.collective_compute(
                "AllGather", mybir.AluOpType.bypass,
                replica_groups=[[0, 1, 2, 3], [4, 5, 6, 7]],
                ins=[src_t.opt()], outs=[dst_t.opt()])

        # ---------------- attention pair machinery ----------------
        sp_map = {}
        zps_map = {}

        def emit_S(pair):
            j, h, p, npairs = pair
            sp = ps_pool.tile([128, 2, QC], F32, tag="ps",
                              name=f"sp{j}_{h}_{p}")
            for u in range(2):
                i = 2 * p + u
                t = i - 4 * j
                qq0 = max(0, t) * 128
                nc.tensor.matmul(
                    sp[:, u, qq0:QC],
                    ktz_c[i // 4][:, h, (i % 4) * 128:(i % 4 + 1) * 128],
                    qt_c[j][:, h // 2, qq0:QC],
                    start=True, stop=True)
            sp_map[(j, h, p)] = sp

        def emit_EZ(idx, pair):
            j, h, p, npairs = pair
            nkt_j = 4 * j + 4
            sp = sp_map.pop((j, h, p))
            pt = ptpool.tile([128, 2, QC], BF16, tag="pt",
                             name=f"pt{j}_{h}_{p}")
            # skip the fully-masked columns below the diagonal block row
            qq0m = max(0, 2 * p - 4 * j) * 128
            nc.scalar.activation(pt[:, :, qq0m:QC], sp[:, :, qq0m:QC],
                                 AF.Exp, bias=0.0, scale=INV_SCALE)
            for u in range(2):
                t = 2 * p + u - 4 * j
                if t >= 0:
                    blk = pt[:, u, 128 * t:128 * (t + 1)]
                    nc.vector.tensor_mul(blk, blk, triu_sb[:])
            if p == 0:
                zps_map[(j, h)] = pr_pool.tile([128, QC], F32, tag="pr",
                                               name=f"zps{j}_{h}")
            zps = zps_map[(j, h)]
            for u in range(2):
                i = 2 * p + u
                qq0 = max(0, i - 4 * j) * 128
                nc.tensor.matmul(
                    zps[0:128, qq0:QC],
                    v_c[i // 4][:, i % 4, h * 65:h * 65 + 128],
                    pt[:, u, qq0:QC],
                    start=(i == 0), stop=(i == nkt_j - 1))
            if p == npairs - 1:
                emit_norm(idx, j, h, zps_map.pop((j, h)))

        # Norms are split into three steps spread over subsequent pairs so
        # the in-order DVE queue never sits in a semaphore wait for the r
        # round-trip DMAs (which can be delayed several us when a
        # collective's wire traffic occupies the DMA engines).
        pending_norm = []   # (due_idx, fn)

        def emit_norm(idx, j, h, zps):
            # step 0 (now): evacuate PSUM, start the r round trip
            zfull = rpool.tile([65, QC], F32, tag="zfull")
            nc.vector.tensor_copy(zfull[:], zps[0:65, :])
            rd = r_dram[j * HLOC + h]
            nc.sync.dma_start(rd[:], zfull[64:65, :])
            rq = rpool.tile([64, 8], F32, tag="rq")
            nc.sync.dma_start(rq[:], rd.rearrange("a (p c) -> (a p) c", p=64))

            def step1():
                nc.vector.reciprocal(rq[:], rq[:])
                rd2 = r_dram2[j * HLOC + h]
                nc.sync.dma_start(rd2.rearrange("a (p c) -> (a p) c", p=64),
                                  rq[:])
                rb = rpool.tile([128, QC], F32, tag="rb")
                nc.sync.dma_start(rb[0:64, :], rd2.to_broadcast((64, QC)))

                def step2():
                    zt_t = ztpool.tile([64, QC], BF16, tag="zt")
                    nc.vector.tensor_mul(zt_t[:], zfull[0:64, :], rb[0:64, :])
                    if j < 3:
                        nc.sync.dma_start(zt_b[j][h * 64:(h + 1) * 64, :],
                                          zt_t[:])
                        if h == HLOC - 1:
                            emit_gather(zt_b[j], zt_all[j])
                    else:
                        nc.sync.dma_start(
                            ztb3[h // 2][(h % 2) * 64:(h % 2 + 1) * 64, :],
                            zt_t[:])
                        if h % 2 == 1:
                            emit_gather(ztb3[h // 2], zta3[h // 2])
                pending_norm.append((idx + 4, step2))
            pending_norm.append((idx + 2, step1))

        # ---------------- output projection ----------------
        za_live = {}

        def za_prefetch(tag, src, src_rows):
            # issue the gathered-Z loads ahead of the matmul lump
            tiles = []
            for r in src_rows:
                za = zapool.tile([128, QC], BF16, tag="za")
                nc.gpsimd.dma_start(za[:], src[r * 128:(r + 1) * 128, :])
                tiles.append(za)
            za_live[tag] = tiles

        def oproj_start(j):
            return [pr_pool.tile([128, QC], F32, tag="pr", name=f"ops{j}_{n}")
                    for n in range(2)]

        def oproj_cdx(ops, cdx_list, tag, first, last):
            tiles = za_live.pop(tag)
            for k, cdx in enumerate(cdx_list):
                for n in range(2):
                    nc.tensor.matmul(
                        ops[n][:],
                        wo_sb[:, cdx, n * 128:(n + 1) * 128],
                        tiles[k][:],
                        start=(first and k == 0),
                        stop=(last and k == len(cdx_list) - 1))

        def oproj_finish(j, ops):
            for n in range(2):
                ot = opool.tile([128, QC], F32, tag="ot")
                nc.vector.tensor_scalar_add(ot[:], ops[n][:],
                                            bo_sb[:, n:n + 1])
                nc.sync.dma_start(
                    out_d[n * 128:(n + 1) * 128, j * QC:(j + 1) * QC], ot[:])

        def oproj_full(j):
            ops = oproj_start(j)
            oproj_cdx(ops, list(range(MC)), ("full", j), True, True)
            oproj_finish(j, ops)

        # ---------------- emission schedule ----------------
        # chunk j pairs: 4 heads x (2j+2) k-tile pairs
        def chunk_pairs(j):
            npairs = 2 * j + 2
            return [(j, h, p, npairs) for h in range(HLOC)
                    for p in range(npairs)]

        # interleave plan: dict pair-stream-position -> list of callables
        all_pairs = []
        inter = {}

        def add_inter(pos, fn):
            inter.setdefault(pos, []).append(fn)

        base = 0
        for j in range(NQ):
            cp = chunk_pairs(j)
            P = len(cp)
            if j == 0:
                # chunk 0 projections run up front
                pass
            if j < NQ - 1:
                # spread chunk j+1's 8 projection groups over 25%..75%
                gs = proj_groups(j + 1)
                for k, g in enumerate(gs):
                    pos = base + int(P * 0.25) + int(k * (P * 0.5) / len(gs))
                    add_inter(pos, g)
            # O-projections all trail the attention stream: cores launch
            # with tens of us of skew, so a mid-stream za-load wait on a
            # collective can freeze the in-order PE queue.  Done at the
            # end, the O-projections are the useful work the leader core
            # performs while the stragglers catch up on the final gather.
            all_pairs.extend(cp)
            base += P

        emitted_S = 0

        def maybe_emit_S(idx):
            nonlocal emitted_S
            if idx < len(all_pairs) and emitted_S <= idx:
                emit_S(all_pairs[idx])
                emitted_S = idx + 1

        def flush_pending(idx):
            todo = [x for x in pending_norm if x[0] <= idx]
            pending_norm[:] = [x for x in pending_norm if x[0] > idx]
            for _, fn in todo:
                fn()

        for g in proj_groups(0):
            g()
        emit_S(all_pairs[0])
        emitted_S = 1
        maybe_emit_S(1)
        for idx in range(len(all_pairs)):
            for fn in inter.get(idx, []):
                fn()
            maybe_emit_S(idx + 1)
            maybe_emit_S(idx + 2)
            emit_EZ(idx, all_pairs[idx])
            flush_pending(idx)
        while pending_norm:
            flush_pending(10 ** 9)

        # trailing: all O-projections.  Gathers 0-2 completed long ago;
        # 3a/3b run concurrently with the j<3 O-projections.
        for j in range(3):
            za_prefetch(("full", j), zt_all[j], list(range(MC)))
            oproj_full(j)
        ops = oproj_start(3)
        za_prefetch(("a3", 0), zta3[0], list(range(4)))
        oproj_cdx(ops, [0, 2, 4, 6], ("a3", 0), True, False)
        za_prefetch(("a3", 1), zta3[1], list(range(4)))
        oproj_cdx(ops, [1, 3, 5, 7], ("a3", 1), False, True)
        oproj_finish(3, ops)

    nc.compile()
    return nc


def _prep_inputs(x, W_Q, W_K, W_V, W_O, b_Q, b_K, b_V, b_O, mask):
    x = np.asarray(x, dtype=np.float32)
    W_Q = np.asarray(W_Q, dtype=np.float32)
    W_K = np.asarray(W_K, dtype=np.float32)
    W_V = np.asarray(W_V, dtype=np.float32)
    W_O = np.asarray(W_O, dtype=np.float32)
    b_Q = np.asarray(b_Q, dtype=np.float32)
    b_K = np.asarray(b_K, dtype=np.float32)
    b_O = np.asarray(b_O, dtype=np.float32)
    b_V = np.asarray(b_V, dtype=np.float32)
    mask = np.asarray(mask)

    # effective output bias: b_O + sum_h W_O[h] @ b_V[h]
    bo_eff = b_O + np.einsum("hnd,hd->n", W_O.astype(np.float64),
                             b_V.astype(np.float64)).astype(np.float32)
    # diagonal 128x128 block of the mask, transposed to (k, q); the kernel
    # skips all fully-masked blocks assuming causal structure
    triu = np.ascontiguousarray(mask[0:128, 0:128].T.astype(np.float32))
    # W^T packs: [m, h*64+d]
    wqT = np.ascontiguousarray(W_Q.transpose(2, 0, 1).reshape(D, H * DH))
    wkT = np.ascontiguousarray(W_K.transpose(2, 0, 1).reshape(D, H * DH))
    wvT = np.ascontiguousarray(W_V.transpose(2, 0, 1).reshape(D, H * DH))
    woT = np.ascontiguousarray(W_O.transpose(0, 2, 1).reshape(H * DH, D))

    in_maps = []
    for c in range(NCORES):
        b = c // 4
        g = c % 4
        hs = slice(4 * g * DH, 4 * (g + 1) * DH)
        bqk = np.stack([
            np.concatenate([b_Q[4 * g], b_Q[4 * g + 1]]),
            np.concatenate([b_Q[4 * g + 2], b_Q[4 * g + 3]]),
            np.concatenate([b_K[4 * g], b_K[4 * g + 1]]),
            np.concatenate([b_K[4 * g + 2], b_K[4 * g + 3]]),
        ], axis=1)
        in_maps.append({
            "xT": np.ascontiguousarray(x[b].T).astype(ml_dtypes.bfloat16),
            "wq": np.ascontiguousarray(wqT[:, hs]).astype(ml_dtypes.bfloat16),
            "wk": np.ascontiguousarray(wkT[:, hs]).astype(ml_dtypes.bfloat16),
            "wv": np.ascontiguousarray(wvT[:, hs]).astype(ml_dtypes.bfloat16),
            "wo": np.ascontiguousarray(
                woT[:, NSL * g:NSL * (g + 1)]).astype(ml_dtypes.bfloat16),
            "bqk": np.ascontiguousarray(bqk.astype(np.float32)),
            "bo": np.ascontiguousarray(
                bo_eff[NSL * g:NSL * (g + 1)].reshape(2, 128).T),
            "triu": triu.astype(ml_dtypes.bfloat16),
        })
    return in_maps


last_exec_time_ns = None


def kernel(x, W_Q, W_K, W_V, W_O, b_Q, b_K, b_V, b_O, mask):
    global last_exec_time_ns
    in_maps = _prep_inputs(x, W_Q, W_K, W_V, W_O, b_Q, b_K, b_V, b_O, mask)
    if "nc" not in _cache:
        _cache["nc"] = _build()
    nc = _cache["nc"]

    trace = os.environ.get("KERNEL_TRACE") == "1"
    if trace:
        import sys, types
        import trn_agent_boot.trn_boot as _tb
        hook = _tb._ntff_profile_via_ctypes('/opt/axon/libaxon_pjrt.so')
        mod = types.ModuleType("antenv.axon_hooks")
        mod.get_axon_ntff_profile_hook = lambda: hook
        mod.set_axon_ntff_profile_hook = lambda h: None
        sys.modules["antenv.axon_hooks"] = mod
        bass_utils.upload_artifacts = lambda tmpdir: f"local:{tmpdir}"

    res = bass_utils.run_bass_kernel_spmd(
        nc, in_maps, core_ids=list(range(NCORES)), trace=trace)
    last_exec_time_ns = res.exec_time_ns
    _cache["last_res"] = res

    out = np.empty((B, S, D), dtype=np.float32)
    for c in range(NCORES):
        b = c // 4
        g = c % 4
        out[b, :, NSL * g:NSL * (g + 1)] = res.results[c]["outT"].T
    return out


# revision 47
# speedup vs baseline: 1.0450x; 1.0450x over previous
"""Multi-head causal attention on 8 Trainium2 NeuronCores.

Problem: B=2, S=2048, D=1024, H=16, DH=64, causal mask, f32.

Sharding: core c -> (batch b = c//4, head group g = c%4 of 4 heads).
Each core computes Q/K/V projections for its 4 heads, streamed causal
attention, AllGathers Z across the 4 cores of a batch group per q-chunk,
and computes a 256-column slice of the output projection.  Host
concatenates slices.

v2 schedule: q-chunks processed ASCENDING (j=0 first) so the exp stream
starts ~20us in and each chunk's AllGather is issued as early as
possible.  Projections for chunk j+1 and the O-projection for chunk j-1
are interleaved into chunk j's attention pair stream so the PE never
starves while ACT (exp) is the constraint.  The final chunk's gather is
split in two (by head pair) so only ~half of it is exposed as tail
latency.  Softmax 1/r broadcast runs on the idle GpSimd engine instead
of a 4-DMA round trip through DRAM.
"""
import os
import numpy as np
import ml_dtypes
from contextlib import ExitStack

import concourse.bacc as bacc
import concourse.tile as tile
from concourse import mybir
from concourse import bass_utils

F32 = mybir.dt.float32
BF16 = mybir.dt.bfloat16
AF = mybir.ActivationFunctionType

B, S, D, H, DH = 2, 2048, 1024, 16, 64
NCORES = 8
HLOC = 4            # heads per core
QC = 512            # q chunk width
NQ = S // QC        # 4 q chunks
KT = 128            # k tile height
NKT = S // KT       # 16 k tiles
MC = D // 128       # 8 contraction chunks
NSL = D // 4        # 256 output columns per core
INV_SCALE = 1.0 / float(np.sqrt(DH))

_cache = {}


def _build():
    nc = bacc.Bacc("TRN2", target_bir_lowering=False, debug=False,
                   num_devices=NCORES)

    xT_d = nc.dram_tensor("xT", [D, S], BF16, kind="ExternalInput").ap()
    wq_d = nc.dram_tensor("wq", [D, 256], BF16, kind="ExternalInput").ap()
    wk_d = nc.dram_tensor("wk", [D, 256], BF16, kind="ExternalInput").ap()
    wv_d = nc.dram_tensor("wv", [D, 256], BF16, kind="ExternalInput").ap()
    wo_d = nc.dram_tensor("wo", [D, NSL], BF16, kind="ExternalInput").ap()
    bqk_d = nc.dram_tensor("bqk", [128, 4], F32, kind="ExternalInput").ap()
    bo_d = nc.dram_tensor("bo", [128, 2], F32, kind="ExternalInput").ap()
    triu_d = nc.dram_tensor("triu", [128, 128], BF16, kind="ExternalInput").ap()
    out_d = nc.dram_tensor("outT", [NSL, S], F32, kind="ExternalOutput").ap()

    with tile.TileContext(nc) as tc, ExitStack() as ctx:
        singles = ctx.enter_context(tc.tile_pool(name="singles", bufs=1))
        ptpool = ctx.enter_context(tc.tile_pool(name="pt", bufs=3))
        ztpool = ctx.enter_context(tc.tile_pool(name="zt", bufs=6))
        rpool = ctx.enter_context(tc.tile_pool(name="rp", bufs=12))
        opool = ctx.enter_context(tc.tile_pool(name="op", bufs=2))
        zapool = ctx.enter_context(tc.tile_pool(name="za", bufs=10))
        ps_pool = ctx.enter_context(tc.tile_pool(name="ps", bufs=3, space="PSUM"))
        pr_pool = ctx.enter_context(tc.tile_pool(name="pr", bufs=2, space="PSUM"))
        dram = ctx.enter_context(tc.tile_pool(name="dram", bufs=1, space="DRAM"))

        # ---------------- persistent SBUF tensors ----------------
        xt_sb = singles.tile([128, MC, S], BF16)      # x[b].T, m-chunked
        wq_sb = singles.tile([128, MC, 256], BF16)
        wk_sb = singles.tile([128, MC, 256], BF16)
        wv_sb = singles.tile([128, MC, 256], BF16)
        wo_sb = singles.tile([128, MC, NSL], BF16)
        bqk_sb = singles.tile([128, 4], F32)
        bo_sb = singles.tile([128, 2], F32)
        triu_sb = singles.tile([128, 128], BF16)
        ones_col = singles.tile([128, 1], F32)
        # Per-chunk tensors (separate tiles -> clean dependency tracking)
        # Q^T chunk: [128, hp, 512]; head pairs packed in partitions.
        qt_c = [singles.tile([128, 2, QC], BF16, name=f"qt{j}")
                for j in range(NQ)]
        # K^T zero-padded per head: even head h keeps rows 0..63 (rest 0),
        # odd head keeps rows 64..127, so the stationary is [128,128] and
        # the PE pipelines LDWEIGHTS (FWL) instead of serializing it.
        ktz_c = [singles.tile([128, HLOC, QC], BF16, name=f"ktz{j}")
                 for j in range(NQ)]
        # V' layout per ktile: [V_h | 1] x 4 heads (65 cols each), padded to
        # 324 so every head can present a [128, 128] stationary slice
        v_c = [singles.tile([128, 4, 324], BF16, name=f"v{j}")
               for j in range(NQ)]

        # ---------------- input DMAs ----------------
        # Keep the scalar (ACT) queue empty so the exp stream starts early;
        # spread issues across sync and gpsimd.
        # The sync queue carries only the small weight loads (so wk/wq land
        # in ~3us and the K/Q projections start immediately); the bulky x^T
        # stream goes on the otherwise-idle gpsimd queue, column-chunk-major
        # (chunk jj's projections need only columns [jj*QC, (jj+1)*QC) of
        # every m-chunk, so the first 4 DMAs unblock chunk 0).
        xT_r = xT_d.rearrange("(c p) q -> p c q", p=128)
        nc.sync.dma_start(wk_sb[:],
                          wk_d.rearrange("(c p) hd -> p c hd", p=128))
        nc.sync.dma_start(bqk_sb[:], bqk_d[:])
        nc.scalar.dma_start(wq_sb[:],
                            wq_d.rearrange("(c p) hd -> p c hd", p=128))
        nc.scalar.dma_start(triu_sb[:], triu_d[:])
        nc.gpsimd.dma_start(wv_sb[:],
                            wv_d.rearrange("(c p) hd -> p c hd", p=128))
        for jj in range(NQ):
            for mc in range(0, MC, 2):
                nc.gpsimd.dma_start(
                    xt_sb[:, mc:mc + 2, jj * QC:(jj + 1) * QC],
                    xT_r[:, mc:mc + 2, jj * QC:(jj + 1) * QC])
        nc.gpsimd.dma_start(bo_sb[:], bo_d[:])
        nc.gpsimd.dma_start(wo_sb[:],
                            wo_d.rearrange("(c p) n -> p c n", p=128))

        # PE warm-up: the first projections are DMA-paced and too sparse to
        # lift the HAM clock gate out of its cold 1.2 GHz state, so burn a
        # dense burst of dummy matmuls while x^T streams in.  ~40 N=128
        # matmuls ≈ 4us of sustained PE-busy -> K=8/8 before K-proj starts.
        warm_sb = singles.tile([128, 128], BF16)
        nc.vector.memset(warm_sb[:], 0.0)
        warm_ps = pr_pool.tile([128, 128], F32, tag="pr", name="warm")
        for _ in range(40):
            nc.tensor.matmul(warm_ps[:], warm_sb[:], warm_sb[:],
                             start=True, stop=True)

        nc.vector.memset(ones_col[:], 1.0)
        memset_done = set()

        def emit_memsets(jj):
            # per-chunk ktz zero-padding + V ones column; emitted just
            # ahead of chunk jj's projections so the DVE queue isn't a
            # 12us wall of memsets before the first bias-add.
            if jj in memset_done:
                return
            memset_done.add(jj)
            nc.vector.memset(v_c[jj][:], 0.0)
            for h in range(HLOC):
                if h % 2 == 0:
                    nc.vector.memset(ktz_c[jj][64:128, h, :], 0.0)
                else:
                    nc.vector.memset(ktz_c[jj][0:64, h, :], 0.0)
            ones_v = v_c[jj][:, :, 0:260].rearrange("p k (h c) -> p k h c",
                                                    c=65)
            for kt in range(4):
                nc.vector.tensor_copy(ones_v[:, kt, :, 64],
                                      ones_col[:, 0:1].to_broadcast((128, 4)))

        # ---------------- projection group emitters ----------------
        # Each projection group is split into two 4-matmul halves so the
        # in-order PE queue never blocks the attention stream for more than
        # ~1us at a time.
        pp_live = {}

        def emit_qk_group(w_sb, is_k, bcol, hp, jj, half):
            # one [128, 512] output chunk of Q^T or K^T (2 heads packed)
            key = (is_k, hp, jj)
            if half == 0:
                pp_live[key] = pr_pool.tile([128, QC], F32, tag="pr",
                                            name=f"pp{int(is_k)}{hp}{jj}")
            pp = pp_live[key]
            for mc in range(4 * half, 4 * half + 4):
                nc.tensor.matmul(
                    pp[:],
                    w_sb[:, mc, hp * 128:(hp + 1) * 128],
                    xt_sb[:, mc, jj * QC:(jj + 1) * QC],
                    start=(mc == 0), stop=(mc == MC - 1))
            if half == 0:
                return
            del pp_live[key]
            if not is_k:
                nc.vector.tensor_scalar_add(
                    qt_c[jj][:, hp, :],
                    pp[:], bqk_sb[:, bcol + hp:bcol + hp + 1])
            else:
                nc.vector.tensor_scalar_add(
                    ktz_c[jj][0:64, 2 * hp, :],
                    pp[0:64, :], bqk_sb[0:64, bcol + hp:bcol + hp + 1])
                nc.vector.tensor_scalar_add(
                    ktz_c[jj][64:128, 2 * hp + 1, :],
                    pp[64:128, :],
                    bqk_sb[64:128, bcol + hp:bcol + hp + 1])

        def emit_v_group(i, half):
            # V[k, hd] for k tile i (natural layout; bias folded into b_O)
            jj, kt = i // 4, i % 4
            if half == 0:
                pp_live[("v", i)] = pr_pool.tile([128, 256], F32, tag="pr",
                                                 name=f"vp{i}")
            vp = pp_live[("v", i)]
            for mc in range(4 * half, 4 * half + 4):
                nc.tensor.matmul(
                    vp[:],
                    xt_sb[:, mc, i * 128:(i + 1) * 128],
                    wv_sb[:, mc, :],
                    start=(mc == 0), stop=(mc == MC - 1))
            if half == 0:
                return
            del pp_live[("v", i)]
            ones_v = v_c[jj][:, :, 0:260].rearrange("p k (h c) -> p k h c", c=65)
            nc.vector.tensor_copy(
                ones_v[:, kt, :, 0:64],
                vp[:].rearrange("p (h c) -> p h c", c=64))

        def proj_groups(jj):
            gs = [lambda: emit_memsets(jj)]
            for hp in range(2):
                for half in range(2):
                    gs.append(lambda hp=hp, half=half:
                              emit_qk_group(wk_sb, True, 2, hp, jj, half))
            for hp in range(2):
                for half in range(2):
                    gs.append(lambda hp=hp, half=half:
                              emit_qk_group(wq_sb, False, 0, hp, jj, half))
            for i in range(4 * jj, 4 * jj + 4):
                for half in range(2):
                    gs.append(lambda i=i, half=half: emit_v_group(i, half))
            return gs

        # ---------------- DRAM staging for Z + collectives ----------------
        zt_b = [dram.tile([HLOC * 64, QC], BF16, name=f"ztb{j}")
                for j in range(3)]
        zt_all = [dram.tile([H * 64, QC], BF16, name=f"zta{j}")
                  for j in range(3)]
        ztb3 = [dram.tile([128, QC], BF16, name=f"ztb3{u}") for u in range(2)]
        zta3 = [dram.tile([512, QC], BF16, name=f"zta3{u}") for u in range(2)]
        r_dram = [dram.tile([1, QC], F32, name=f"rd{j}_{h}")
                  for j in range(NQ) for h in range(HLOC)]
        r_dram2 = [dram.tile([1, QC], F32, name=f"re{j}_{h}")
                   for j in range(NQ) for h in range(HLOC)]

        def emit_gather(src_t, dst_t):
            nc.gpsimd.collective_compute(
                "AllGather", mybir.AluOpType.bypass,
                replica_groups=[[0, 1, 2, 3], [4, 5, 6, 7]],
                ins=[src_t.opt()], outs=[dst_t.opt()])

        # ---------------- attention pair machinery ----------------
        sp_map = {}
        zps_map = {}

        def emit_S(pair):
            j, h, p, npairs = pair
            sp = ps_pool.tile([128, 2, QC], F32, tag="ps",
                              name=f"sp{j}_{h}_{p}")
            for u in range(2):
                i = 2 * p + u
                t = i - 4 * j
                qq0 = max(0, t) * 128
                nc.tensor.matmul(
                    sp[:, u, qq0:QC],
                    ktz_c[i // 4][:, h, (i % 4) * 128:(i % 4 + 1) * 128],
                    qt_c[j][:, h // 2, qq0:QC],
                    start=True, stop=True)
            sp_map[(j, h, p)] = sp

        def emit_EZ(idx, pair):
            j, h, p, npairs = pair
            nkt_j = 4 * j + 4
            sp = sp_map.pop((j, h, p))
            pt = ptpool.tile([128, 2, QC], BF16, tag="pt",
                             name=f"pt{j}_{h}_{p}")
            # skip the fully-masked columns below the diagonal block row
            qq0m = max(0, 2 * p - 4 * j) * 128
            nc.scalar.activation(pt[:, :, qq0m:QC], sp[:, :, qq0m:QC],
                                 AF.Exp, bias=0.0, scale=INV_SCALE)
            for u in range(2):
                t = 2 * p + u - 4 * j
                if t >= 0:
                    blk = pt[:, u, 128 * t:128 * (t + 1)]
                    nc.vector.tensor_mul(blk, blk, triu_sb[:])
            if p == 0:
                zps_map[(j, h)] = pr_pool.tile([128, QC], F32, tag="pr",
                                               name=f"zps{j}_{h}")
            zps = zps_map[(j, h)]
            for u in range(2):
                i = 2 * p + u
                qq0 = max(0, i - 4 * j) * 128
                nc.tensor.matmul(
                    zps[0:128, qq0:QC],
                    v_c[i // 4][:, i % 4, h * 65:h * 65 + 128],
                    pt[:, u, qq0:QC],
                    start=(i == 0), stop=(i == nkt_j - 1))
            if p == npairs - 1:
                emit_norm(idx, j, h, zps_map.pop((j, h)))

        # Norms are split into three steps spread over subsequent pairs so
        # the in-order DVE queue never sits in a semaphore wait for the r
        # round-trip DMAs (which can be delayed several us when a
        # collective's wire traffic occupies the DMA engines).
        pending_norm = []   # (due_idx, fn)

        def emit_norm(idx, j, h, zps):
            # step 0 (now): evacuate PSUM, start the r round trip
            zfull = rpool.tile([65, QC], F32, tag="zfull")
            nc.vector.tensor_copy(zfull[:], zps[0:65, :])
            rd = r_dram[j * HLOC + h]
            nc.sync.dma_start(rd[:], zfull[64:65, :])
            rq = rpool.tile([64, 8], F32, tag="rq")
            nc.sync.dma_start(rq[:], rd.rearrange("a (p c) -> (a p) c", p=64))

            def step1():
                nc.vector.reciprocal(rq[:], rq[:])
                rd2 = r_dram2[j * HLOC + h]
                nc.sync.dma_start(rd2.rearrange("a (p c) -> (a p) c", p=64),
                                  rq[:])
                rb = rpool.tile([128, QC], F32, tag="rb")
                nc.sync.dma_start(rb[0:64, :], rd2.to_broadcast((64, QC)))

                def step2():
                    zt_t = ztpool.tile([64, QC], BF16, tag="zt")
                    nc.vector.tensor_mul(zt_t[:], zfull[0:64, :], rb[0:64, :])
                    if j < 3:
                        nc.sync.dma_start(zt_b[j][h * 64:(h + 1) * 64, :],
                                          zt_t[:])
                        if h == HLOC - 1:
                            emit_gather(zt_b[j], zt_all[j])
                    else:
                        nc.sync.dma_start(
                            ztb3[h // 2][(h % 2) * 64:(h % 2 + 1) * 64, :],
                            zt_t[:])
                        if h == 1:
                            emit_gather(ztb3[0], zta3[0])
                        elif h == 3:
                            # the final gather is triggered from the
                            # trailing section, after the O-projection za
                            # loads have been issued on the same queue
                            _cache["fin_g"] = lambda: emit_gather(ztb3[1],
                                                                  zta3[1])
                pending_norm.append((idx + 4, step2))
            pending_norm.append((idx + 2, step1))

        # ---------------- output projection ----------------
        za_live = {}

        def za_prefetch(tag, src, src_rows):
            # issue the gathered-Z loads ahead of the matmul lump
            tiles = []
            for r in src_rows:
                za = zapool.tile([128, QC], BF16, tag="za")
                nc.gpsimd.dma_start(za[:], src[r * 128:(r + 1) * 128, :])
                tiles.append(za)
            za_live[tag] = tiles

        def oproj_start(j):
            return [pr_pool.tile([128, QC], F32, tag="pr", name=f"ops{j}_{n}")
                    for n in range(2)]

        def oproj_cdx(ops, cdx_list, tag, first, last):
            tiles = za_live.pop(tag)
            for k, cdx in enumerate(cdx_list):
                for n in range(2):
                    nc.tensor.matmul(
                        ops[n][:],
                        wo_sb[:, cdx, n * 128:(n + 1) * 128],
                        tiles[k][:],
                        start=(first and k == 0),
                        stop=(last and k == len(cdx_list) - 1))

        def oproj_finish(j, ops):
            for n in range(2):
                ot = opool.tile([128, QC], F32, tag="ot")
                nc.vector.tensor_scalar_add(ot[:], ops[n][:],
                                            bo_sb[:, n:n + 1])
                nc.sync.dma_start(
                    out_d[n * 128:(n + 1) * 128, j * QC:(j + 1) * QC], ot[:])

        def oproj_full(j):
            ops = oproj_start(j)
            oproj_cdx(ops, list(range(MC)), ("full", j), True, True)
            oproj_finish(j, ops)

        # ---------------- emission schedule ----------------
        # chunk j pairs: 4 heads x (2j+2) k-tile pairs
        def chunk_pairs(j):
            npairs = 2 * j + 2
            return [(j, h, p, npairs) for h in range(HLOC)
                    for p in range(npairs)]

        # interleave plan: dict pair-stream-position -> list of callables
        all_pairs = []
        inter = {}

        def add_inter(pos, fn):
            inter.setdefault(pos, []).append(fn)

        base = 0
        for j in range(NQ):
            cp = chunk_pairs(j)
            P = len(cp)
            if j == 0:
                # chunk 0 projections run up front
                pass
            if j < NQ - 1:
                # spread chunk j+1's 8 projection groups over 25%..75%
                gs = proj_groups(j + 1)
                for k, g in enumerate(gs):
                    pos = base + int(P * 0.25) + int(k * (P * 0.5) / len(gs))
                    add_inter(pos, g)
            # O-projections all trail the attention stream: cores launch
            # with tens of us of skew, so a mid-stream za-load wait on a
            # collective can freeze the in-order PE queue.  Done at the
            # end, the O-projections are the useful work the leader core
            # performs while the stragglers catch up on the final gather.
            all_pairs.extend(cp)
            base += P

        emitted_S = 0

        def maybe_emit_S(idx):
            nonlocal emitted_S
            if idx < len(all_pairs) and emitted_S <= idx:
                emit_S(all_pairs[idx])
                emitted_S = idx + 1

        def flush_pending(idx):
            todo = [x for x in pending_norm if x[0] <= idx]
            pending_norm[:] = [x for x in pending_norm if x[0] > idx]
            for _, fn in todo:
                fn()

        for g in proj_groups(0):
            g()
        emit_S(all_pairs[0])
        emitted_S = 1
        maybe_emit_S(1)
        for idx in range(len(all_pairs)):
            for fn in inter.get(idx, []):
                fn()
            maybe_emit_S(idx + 1)
            maybe_emit_S(idx + 2)
            emit_EZ(idx, all_pairs[idx])
            flush_pending(idx)
        while pending_norm:
            flush_pending(10 ** 9)

        # trailing: all O-projections.  Gathers 0-2 completed long ago and
        # 3a lands while the tail pairs run, so the leader core spends the
        # final-gather wait on useful matmuls.  The final gather trigger is
        # emitted after the za issues so its semaphore wait doesn't block
        # them on the in-order gpsimd queue.
        for j in range(3):
            za_prefetch(("full", j), zt_all[j], list(range(MC)))
            oproj_full(j)
        ops = oproj_start(3)
        za_prefetch(("a3", 0), zta3[0], list(range(4)))
        oproj_cdx(ops, [0, 2, 4, 6], ("a3", 0), True, False)
        _cache.pop("fin_g")()
        za_prefetch(("a3", 1), zta3[1], list(range(4)))
        oproj_cdx(ops, [1, 3, 5, 7], ("a3", 1), False, True)
        oproj_finish(3, ops)

    nc.compile()
    return nc


def _prep_inputs(x, W_Q, W_K, W_V, W_O, b_Q, b_K, b_V, b_O, mask):
    x = np.asarray(x, dtype=np.float32)
    W_Q = np.asarray(W_Q, dtype=np.float32)
    W_K = np.asarray(W_K, dtype=np.float32)
    W_V = np.asarray(W_V, dtype=np.float32)
    W_O = np.asarray(W_O, dtype=np.float32)
    b_Q = np.asarray(b_Q, dtype=np.float32)
    b_K = np.asarray(b_K, dtype=np.float32)
    b_O = np.asarray(b_O, dtype=np.float32)
    b_V = np.asarray(b_V, dtype=np.float32)
    mask = np.asarray(mask)

    # effective output bias: b_O + sum_h W_O[h] @ b_V[h]
    bo_eff = b_O + np.einsum("hnd,hd->n", W_O.astype(np.float64),
                             b_V.astype(np.float64)).astype(np.float32)
    # diagonal 128x128 block of the mask, transposed to (k, q); the kernel
    # skips all fully-masked blocks assuming causal structure
    triu = np.ascontiguousarray(mask[0:128, 0:128].T.astype(np.float32))
    # W^T packs: [m, h*64+d]
    wqT = np.ascontiguousarray(W_Q.transpose(2, 0, 1).reshape(D, H * DH))
    wkT = np.ascontiguousarray(W_K.transpose(2, 0, 1).reshape(D, H * DH))
    wvT = np.ascontiguousarray(W_V.transpose(2, 0, 1).reshape(D, H * DH))
    woT = np.ascontiguousarray(W_O.transpose(0, 2, 1).reshape(H * DH, D))

    in_maps = []
    for c in range(NCORES):
        b = c // 4
        g = c % 4
        hs = slice(4 * g * DH, 4 * (g + 1) * DH)
        bqk = np.stack([
            np.concatenate([b_Q[4 * g], b_Q[4 * g + 1]]),
            np.concatenate([b_Q[4 * g + 2], b_Q[4 * g + 3]]),
            np.concatenate([b_K[4 * g], b_K[4 * g + 1]]),
            np.concatenate([b_K[4 * g + 2], b_K[4 * g + 3]]),
        ], axis=1)
        in_maps.append({
            "xT": np.ascontiguousarray(x[b].T).astype(ml_dtypes.bfloat16),
            "wq": np.ascontiguousarray(wqT[:, hs]).astype(ml_dtypes.bfloat16),
            "wk": np.ascontiguousarray(wkT[:, hs]).astype(ml_dtypes.bfloat16),
            "wv": np.ascontiguousarray(wvT[:, hs]).astype(ml_dtypes.bfloat16),
            "wo": np.ascontiguousarray(
                woT[:, NSL * g:NSL * (g + 1)]).astype(ml_dtypes.bfloat16),
            "bqk": np.ascontiguousarray(bqk.astype(np.float32)),
            "bo": np.ascontiguousarray(
                bo_eff[NSL * g:NSL * (g + 1)].reshape(2, 128).T),
            "triu": triu.astype(ml_dtypes.bfloat16),
        })
    return in_maps


last_exec_time_ns = None


def kernel(x, W_Q, W_K, W_V, W_O, b_Q, b_K, b_V, b_O, mask):
    global last_exec_time_ns
    in_maps = _prep_inputs(x, W_Q, W_K, W_V, W_O, b_Q, b_K, b_V, b_O, mask)
    if "nc" not in _cache:
        _cache["nc"] = _build()
    nc = _cache["nc"]

    trace = os.environ.get("KERNEL_TRACE") == "1"
    if trace:
        import sys, types
        import trn_agent_boot.trn_boot as _tb
        hook = _tb._ntff_profile_via_ctypes('/opt/axon/libaxon_pjrt.so')
        mod = types.ModuleType("antenv.axon_hooks")
        mod.get_axon_ntff_profile_hook = lambda: hook
        mod.set_axon_ntff_profile_hook = lambda h: None
        sys.modules["antenv.axon_hooks"] = mod
        bass_utils.upload_artifacts = lambda tmpdir: f"local:{tmpdir}"

    res = bass_utils.run_bass_kernel_spmd(
        nc, in_maps, core_ids=list(range(NCORES)), trace=trace)
    last_exec_time_ns = res.exec_time_ns
    _cache["last_res"] = res

    out = np.empty((B, S, D), dtype=np.float32)
    for c in range(NCORES):
        b = c // 4
        g = c % 4
        out[b, :, NSL * g:NSL * (g + 1)] = res.results[c]["outT"].T
    return out
